# revision 23
# baseline (speedup 1.0000x reference)
"""Trainium2 Bass kernel for GQA attention (B=2, S=2048, D=4096, 32 q-heads,
8 kv-heads, head_dim=128, RoPE, causal) distributed over 8 NeuronCores.

Sharding: tensor-parallel over heads for QKV+attention (core c owns q-heads
4c..4c+3 and kv-head c, all sequence positions), then per-head AllToAlls
re-shard the attention output from head-sharded to row-sharded (overlapped
with attention of later heads) so the output projection wo contracts locally;
final output rows are gathered on the host.

Device dataflow per core (identical program on all 8 cores, data differs):
  - stream x^T tiles once; accumulate K^T, V^T (own kv head) and Q^T (4 own
    heads) in PSUM; drain PSUM quickly via ScalarE copies; rotate-half RoPE
    on VectorE (weights are column-permuted on the host so RoPE pairs are
    (i, i+64) within each head); V^T -> V via PE transposes, inline per chunk.
  - causal attention per (head, batch, q-tile of 128), software-pipelined:
    scores in PSUM (PE) -> exp on ScalarE (accumulated row-sum = softmax
    denominator for free; no max subtraction needed at these magnitudes) ->
    normalize P by 1/den (VectorE) -> PE-transpose P -> P^T @ V on PE.
  - AllToAll per head (head-sharded -> row-sharded), overlapped.
  - out^T = wo^T @ attn_out^T for this core's 512 rows.
Host returns out[rows_c, :] = out_c^T.T concatenated over cores.
"""
import sys
sys.path.insert(0, "/opt/trn_rl_repo")
import math
import numpy as np

import concourse.bass as bass
import concourse.bacc as bacc
import concourse.tile as tile
import concourse.mybir as mybir
from concourse.bass_utils import run_bass_kernel_spmd
from concourse.dt import dt

B, S, D = 2, 2048, 4096
HQ, HKV, HD = 32, 8, 128
NC_ = 8                       # cores
ROWS = B * S                  # 4096
RPC = ROWS // NC_             # 512 rows per core
HPC = HQ // NC_               # 4 q-heads per core
QT = S // HD                  # 16 q-tiles per batch
DT_ = 32                      # d-tiles (D/128)
SCALE = 1.0 / math.sqrt(HD)
THETA = 10000.0

F32 = mybir.dt.float32
CDT = mybir.dt.bfloat16       # compute dtype for matmul operands
NP_CDT = dt.np(CDT)

_CACHE = {}


def _build():
    nc = bacc.Bacc("TRN2", target_bir_lowering=False, debug=False,
                   num_devices=NC_)
    AF = mybir.ActivationFunctionType

    # ---- dram I/O (names = in_maps keys) ----
    xT = nc.dram_tensor("xT", [DT_, 128, ROWS], CDT, kind="ExternalInput")
    wq = nc.dram_tensor("wq", [128, DT_ * HPC * HD], CDT, kind="ExternalInput")
    wk = nc.dram_tensor("wk", [128, DT_ * HD], CDT, kind="ExternalInput")
    wv = nc.dram_tensor("wv", [128, DT_ * HD], CDT, kind="ExternalInput")
    wo = nc.dram_tensor("wo", [(DT_ // 8) * HPC, 128, NC_ * 8 * HD], CDT,
                        kind="ExternalInput")
    csa = nc.dram_tensor("csa", [128, S], F32, kind="ExternalInput")  # cos|cos
    csb = nc.dram_tensor("csb", [128, S], F32, kind="ExternalInput")  # -sin|sin
    msk = nc.dram_tensor("msk", [128, HD], F32, kind="ExternalInput")
    idn = nc.dram_tensor("idn", [128, 128], CDT, kind="ExternalInput")
    psw = nc.dram_tensor("psw", [128, 128], CDT, kind="ExternalInput")
    out = nc.dram_tensor("out", [DT_ * 128, RPC], F32, kind="ExternalOutput")

    with tile.TileContext(nc) as tc:
        with tc.tile_pool(name="const", bufs=1) as constp, \
             tc.tile_pool(name="persist", bufs=1) as persist, \
             tc.tile_pool(name="dram", bufs=1, space="DRAM") as dram:
            # constants (cos/sin tables loaded on the scalar queue so they
            # don't delay the first projection matmuls' weight DMAs)
            csa_sb = constp.tile([128, S], F32)
            nc.scalar.dma_start(csa_sb[:], csa[:])
            csb_sb = constp.tile([128, S], F32)
            nc.scalar.dma_start(csb_sb[:], csb[:])
            msk_sb = constp.tile([128, HD], F32)
            nc.scalar.dma_start(msk_sb[:], msk[:])
            idn_sb = constp.tile([128, 128], CDT)
            nc.scalar.dma_start(idn_sb[:], idn[:])
            psw_sb = constp.tile([128, 128], CDT)
            nc.scalar.dma_start(psw_sb[:], psw[:])

            # persistent activations
            k_sb = persist.tile([128, ROWS], CDT)           # K^T (rope'd)
            vn_sb = persist.tile([128, ROWS], CDT)          # V natural tiles
            q_sb = persist.tile([128, HPC * ROWS], CDT)     # Q^T per head
            ao_sb = persist.tile([128, HPC * ROWS], CDT)    # attn_out^T per head

            def _rope(dst, src_sb, swp_ps, pos0, n, tmp_pool):
                """dst[128, n] (SBUF CDT) = rope(src_sb[128, n] SBUF CDT),
                given swp_ps[128, n] (PSUM f32) = half-swapped src (from a PE
                matmul with the swap permutation). Positions pos0.. (one
                batch). All ops are full-tile, partition-aligned:
                  dst = src * [cos|cos] + swapped(src) * [-sin|sin]."""
                ca = csa_sb[:, pos0:pos0 + n]
                cb = csb_sb[:, pos0:pos0 + n]
                t = tmp_pool.tile([128, n], F32, tag="ropetmp")
                nc.vector.tensor_mul(t[:], src_sb[:], ca)
                u = tmp_pool.tile([128, n], F32, tag="ropetmp2")
                nc.vector.tensor_mul(u[:], swp_ps[:], cb)
                nc.vector.tensor_add(dst[:], t[:], u[:])

            # ---------------- phase P: projections ----------------
            with tc.tile_pool(name="wts", bufs=1) as wtp, \
                 tc.tile_pool(name="xs", bufs=6) as xsp, \
                 tc.tile_pool(name="ptmp", bufs=2) as ptmp, \
                 tc.tile_pool(name="drain", bufs=2) as drp, \
                 tc.tile_pool(name="pps", bufs=1, space="PSUM") as pps, \
                 tc.tile_pool(name="miscp", bufs=2, space="PSUM") as miscp:
                # weight loads chunked so the first d-tiles' matmuls can
                # start before the full weight tensors land
                wk_sb = wtp.tile([128, DT_ * HD], CDT)
                wv_sb = wtp.tile([128, DT_ * HD], CDT)
                wq_sb = wtp.tile([128, DT_ * HPC * HD], CDT)
                for qd in range(8):
                    c0, c1 = qd * DT_ * HD // 8, (qd + 1) * DT_ * HD // 8
                    nc.sync.dma_start(wk_sb[:, c0:c1], wk[:, c0:c1])
                    nc.sync.dma_start(wv_sb[:, c0:c1], wv[:, c0:c1])
                    q0_, q1_ = qd * DT_ * HPC * HD // 8, \
                        (qd + 1) * DT_ * HPC * HD // 8
                    nc.sync.dma_start(wq_sb[:, q0_:q1_], wq[:, q0_:q1_])

                NCH = ROWS // 512   # 8 row-chunks
                for ch in range(NCH):
                    pos0 = (ch * 512) % S
                    kp = pps.tile([128, 512], F32, tag="kp")
                    vp = pps.tile([128, 512], F32, tag="vp")
                    qp = [pps.tile([128, 512], F32, tag=f"qp{h}",
                                   name=f"qp{h}_{ch}")
                          for h in range(HPC)]
                    for d in range(DT_):
                        xt = xsp.tile([128, 512], CDT, tag="xt")
                        nc.sync.dma_start(
                            xt[:], xT[d, :, ch * 512:(ch + 1) * 512])
                        st, sp = (d == 0), (d == DT_ - 1)
                        nc.tensor.matmul(kp[:], wk_sb[:, d * HD:(d + 1) * HD],
                                         xt[:], start=st, stop=sp)
                        nc.tensor.matmul(vp[:], wv_sb[:, d * HD:(d + 1) * HD],
                                         xt[:], start=st, stop=sp)
                        for h in range(HPC):
                            w0 = (d * HPC + h) * HD
                            nc.tensor.matmul(qp[h][:],
                                             wq_sb[:, w0:w0 + HD],
                                             xt[:], start=st, stop=sp)
                    sl = slice(ch * 512, (ch + 1) * 512)
                    # drain PSUM fast via ScalarE copies (cast to bf16),
                    # half-swap on PE, then rope on DVE
                    kf = drp.tile([128, 512], CDT, tag="kf")
                    nc.scalar.copy(kf[:], kp[:])
                    vf = drp.tile([128, 512], CDT, tag="vf")
                    nc.scalar.copy(vf[:], vp[:])   # V^T chunk (bf16)
                    qf = [drp.tile([128, 512], CDT, tag=f"qf{h}",
                                   name=f"qf{h}_{ch}")
                          for h in range(HPC)]
                    for h in range(HPC):
                        nc.scalar.copy(qf[h][:], qp[h][:])
                    ksw = miscp.tile([128, 512], F32, tag="misc",
                                     name=f"ksw_{ch}")
                    nc.tensor.matmul(ksw[:], psw_sb[:], kf[:],
                                     start=True, stop=True)
                    _rope(k_sb[:, sl], kf[:], ksw[:], pos0, 512, ptmp)
                    for h in range(HPC):
                        qsw = miscp.tile([128, 512], F32, tag="misc",
                                         name=f"qsw{h}_{ch}")
                        nc.tensor.matmul(qsw[:], psw_sb[:], qf[h][:],
                                         start=True, stop=True)
                        _rope(q_sb[:, h * ROWS + ch * 512:
                                   h * ROWS + (ch + 1) * 512],
                              qf[h][:], qsw[:], pos0, 512, ptmp)
                    # V^T -> V natural, inline (PE transposes + DVE drain)
                    vt = miscp.tile([128, 512], CDT, tag="misc",
                                    name=f"vt_{ch}")
                    for t in range(4):
                        nc.tensor.transpose(vt[:, t * 128:(t + 1) * 128],
                                            vf[:, t * 128:(t + 1) * 128],
                                            idn_sb[:])
                    nc.vector.tensor_copy(vn_sb[:, sl], vt[:])

            # ---------------- phase A: attention (+ per-head A2A) --------
            a2a_in = dram.tile([HPC, NC_, 128, RPC], CDT)
            a2a_out = dram.tile([HPC, NC_, 128, RPC], CDT)

            with tc.tile_pool(name="att", bufs=3) as att, \
                 tc.tile_pool(name="attd", bufs=6) as attd, \
                 tc.tile_pool(name="sps", bufs=4, space="PSUM") as sps, \
                 tc.tile_pool(name="tps2", bufs=2, space="PSUM") as tps2, \
                 tc.tile_pool(name="ops", bufs=2, space="PSUM") as ops:

                def stage1(h, b, j):
                    """QK -> exp/den -> normalized P (PE work: QK matmuls)."""
                    klen = HD * (j + 1)
                    q0 = h * ROWS + b * S + j * HD
                    qt_ap = q_sb[:, q0:q0 + HD]
                    nkc = (klen + 511) // 512
                    p_t = att.tile([128, 2048], CDT, tag="p",
                                   name=f"p_{h}_{b}_{j}")
                    den = attd.tile([128, 4], F32, tag="den",
                                    name=f"den_{h}_{b}_{j}")
                    for kc in range(nkc):
                        k0 = kc * 512
                        kl = min(512, klen - k0)
                        sp_ = sps.tile([128, 512], F32, tag="sp",
                                       name=f"sp_{h}_{b}_{j}_{kc}")
                        nc.tensor.matmul(
                            sp_[:, 0:kl], qt_ap,
                            k_sb[:, b * S + k0:b * S + k0 + kl],
                            start=True, stop=True)
                        if kc == nkc - 1:
                            nc.vector.tensor_add(
                                sp_[:, kl - HD:kl], sp_[:, kl - HD:kl],
                                msk_sb[:])
                        nc.scalar.activation(
                            p_t[:, k0:k0 + kl], sp_[:, 0:kl],
                            AF.Exp, scale=SCALE,
                            accum_out=den[:, kc:kc + 1])
                    for kc in range(1, nkc):
                        nc.vector.tensor_add(den[:, 0:1], den[:, 0:1],
                                             den[:, kc:kc + 1])
                    rden = attd.tile([128, 1], F32, tag="rden",
                                     name=f"rden_{h}_{b}_{j}")
                    nc.vector.reciprocal(rden[:], den[:, 0:1])
                    nc.vector.tensor_scalar_mul(p_t[:, 0:klen],
                                                p_t[:, 0:klen], rden[:])
                    return p_t

                def stage2(h, b, j, p_t, otg):
                    """P transpose -> PV accumulate -> drain OT group."""
                    klen = HD * (j + 1)
                    q0 = h * ROWS + b * S + j * HD
                    pt_t = att.tile([128, 2048], CDT, tag="pt",
                                    name=f"pt_{h}_{b}_{j}")
                    for g in range(0, j + 1, 4):
                        gw = min(4, j + 1 - g)
                        tp = tps2.tile([128, 512], CDT, tag="tp",
                                       name=f"tp_{h}_{b}_{j}_{g}")
                        for t in range(gw):
                            c0 = (g + t) * HD
                            nc.tensor.transpose(tp[:, t * HD:(t + 1) * HD],
                                                p_t[:, c0:c0 + HD], idn_sb[:])
                        nc.vector.tensor_copy(pt_t[:, g * HD:(g + gw) * HD],
                                              tp[:, 0:gw * HD])
                    jj = j % 4
                    for kt in range(j + 1):
                        nc.tensor.matmul(
                            otg[:, jj * HD:(jj + 1) * HD],
                            vn_sb[:, (b * QT + kt) * HD:(b * QT + kt + 1) * HD],
                            pt_t[:, kt * HD:(kt + 1) * HD],
                            start=(kt == 0), stop=(kt == j))
                    if jj == 3:
                        # drain 4 accumulated OTs at once
                        nc.vector.tensor_copy(
                            ao_sb[:, q0 - 3 * HD:q0 + HD], otg[:])

                for h in range(HPC):
                    pending = None
                    otg = None
                    for b in range(B):
                        for j in range(QT):
                            if j % 4 == 0:
                                otg = ops.tile([128, 512], F32, tag="ot",
                                               name=f"ot_{h}_{b}_{j}")
                            p_t = stage1(h, b, j)
                            if pending is not None:
                                stage2(*pending)
                            pending = (h, b, j, p_t, otg)
                            otg_prev = otg
                    stage2(*pending)
                    # per-head AllToAll, overlaps later heads' attention
                    for r in range(NC_):
                        nc.sync.dma_start(
                            a2a_in[h, r, :, :],
                            ao_sb[:, h * ROWS + r * RPC:
                                  h * ROWS + (r + 1) * RPC])
                    nc.gpsimd.collective_compute(
                        "AllToAll", mybir.AluOpType.bypass,
                        ins=[a2a_in[h].opt()], outs=[a2a_out[h].opt()],
                        replica_groups=[list(range(NC_))],
                    )

            # ---------------- phase O: output projection ----------------
            # od-groups of 8 (one PSUM bank each) x h-passes: the h-pass MMs
            # only depend on collective h, so the first 3/4 of each group's
            # matmuls can run before the last A2A lands.
            NG = DT_ // 8   # 4 od-groups
            with tc.tile_pool(name="osb", bufs=1) as osb, \
                 tc.tile_pool(name="wos", bufs=2) as wos, \
                 tc.tile_pool(name="oout", bufs=3) as oout, \
                 tc.tile_pool(name="opp", bufs=1, space="PSUM") as opp:
                # ao2 loads issued on the gpsimd queue: it is naturally
                # ordered after each collective, so head-3's load cannot
                # block the wo weight loads on the sync queue.
                ao2 = osb.tile([128, DT_ * RPC], CDT)
                for h in range(HPC):
                    for s_ in range(NC_):
                        ct = s_ * HPC + h
                        nc.gpsimd.dma_start(
                            ao2[:, ct * RPC:(ct + 1) * RPC],
                            a2a_out[h, s_, :, :])
                for g in range(NG):
                    ops_ = [opp.tile([128, RPC], F32, tag=f"op{i}",
                                     name=f"op_{g}_{i}") for i in range(8)]
                    for h in range(HPC):
                        # weights for this (group, h): [128, s, od_in_g, 128]
                        wo_sb = wos.tile([128, NC_ * 8 * HD], CDT, tag="wo",
                                         name=f"wo_{g}_{h}")
                        nc.sync.dma_start(wo_sb[:], wo[g * HPC + h, :, :])
                        for s_ in range(NC_):
                            ct = s_ * HPC + h
                            for i in range(8):
                                w0 = (s_ * 8 + i) * HD
                                nc.tensor.matmul(
                                    ops_[i][:], wo_sb[:, w0:w0 + HD],
                                    ao2[:, ct * RPC:(ct + 1) * RPC],
                                    start=(h == 0 and s_ == 0),
                                    stop=(h == HPC - 1 and s_ == NC_ - 1))
                    for i in range(8):
                        od = g * 8 + i
                        o_sb = oout.tile([128, RPC], F32, tag="osb",
                                         name=f"osb_{od}")
                        nc.scalar.copy(o_sb[:], ops_[i][:])
                        nc.sync.dma_start(out[od * 128:(od + 1) * 128, :],
                                          o_sb[:])

    nc.compile()
    return nc


def _host_prep(x, wq, wk, wv, wo):
    perm = np.concatenate([np.arange(0, HD, 2), np.arange(1, HD, 2)])
    x2 = np.ascontiguousarray(x.reshape(ROWS, D).T)        # [D, ROWS]
    xT_r = x2.reshape(DT_, 128, ROWS).astype(NP_CDT)

    wq_p = wq.reshape(D, HQ, HD)[:, :, perm].reshape(D, HQ * HD)
    wk_p = wk.reshape(D, HKV, HD)[:, :, perm].reshape(D, HKV * HD)

    # per-core weight shards in sbuf tile layout [128p, d-tile, cols]
    def tile_rows(w):  # [D, C] -> [128, DT_*C] with blocks (d, c)
        Dd, C = w.shape
        return np.ascontiguousarray(
            w.reshape(DT_, 128, C).transpose(1, 0, 2).reshape(128, DT_ * C))

    wq_cores = []
    wk_cores = []
    wv_cores = []
    for c in range(NC_):
        wqc = wq_p[:, c * HPC * HD:(c + 1) * HPC * HD]     # [D, 512]
        wq_cores.append(tile_rows(wqc).astype(NP_CDT))
        wk_cores.append(tile_rows(
            wk_p[:, c * HD:(c + 1) * HD]).astype(NP_CDT))
        wv_cores.append(tile_rows(
            wv[:, c * HD:(c + 1) * HD]).astype(NP_CDT))

    # wo lhsT tiles grouped by (od-group g, head-slot h):
    # wo_t[g*HPC+h, p, (s*8+i)*128+j] = wo[(s*HPC+h)*128+p, (g*8+i)*128+j]
    wo_t = np.ascontiguousarray(
        wo.reshape(NC_, HPC, 128, DT_ // 8, 8, 128)
        .transpose(3, 1, 2, 0, 4, 5)
        .reshape((DT_ // 8) * HPC, 128, NC_ * 8 * 128)).astype(NP_CDT)

    inv = 1.0 / (THETA ** (np.arange(0, HD, 2, dtype=np.float64) / HD))
    ang = np.arange(S, dtype=np.float64)[:, None] * inv[None, :]
    cosT = np.cos(ang).T
    sinT = np.sin(ang).T
    csa = np.concatenate([cosT, cosT], axis=0).astype(np.float32)
    csb = np.concatenate([-sinT, sinT], axis=0).astype(np.float32)

    m = np.where(np.arange(HD)[None, :] > np.arange(HD)[:, None],
                 np.float32(-1e9), np.float32(0.0)).astype(np.float32)
    ident = np.eye(128, dtype=np.float32).astype(NP_CDT)
    # half-swap permutation: out[i] = in[(i+64)%128]  (out = psw.T @ in)
    pswap = np.zeros((128, 128), np.float32)
    pswap[(np.arange(128) + 64) % 128, np.arange(128)] = 1.0
    pswap = pswap.astype(NP_CDT)

    return (xT_r, wq_cores, wk_cores, wv_cores, wo_t, csa, csb, m, ident,
            pswap)


def kernel(x, wq, wk, wv, wo):
    if "nc" not in _CACHE:
        _CACHE["nc"] = _build()
    nc = _CACHE["nc"]

    xT_r, wq_c, wk_c, wv_c, wo_t, csa, csb, m, ident, pswap = _host_prep(
        np.asarray(x, np.float32), np.asarray(wq, np.float32),
        np.asarray(wk, np.float32), np.asarray(wv, np.float32),
        np.asarray(wo, np.float32))

    in_maps = []
    for c in range(NC_):
        in_maps.append({
            "xT": xT_r, "wq": wq_c[c], "wk": wk_c[c], "wv": wv_c[c],
            "wo": wo_t, "csa": csa, "csb": csb, "msk": m, "idn": ident,
            "psw": pswap,
        })
    res = run_bass_kernel_spmd(nc, in_maps, core_ids=list(range(NC_)))
    _CACHE["last_results"] = res

    outp = np.empty((ROWS, D), np.float32)
    for c in range(NC_):
        outp[c * RPC:(c + 1) * RPC, :] = res.results[c]["out"].T
    return outp.reshape(B, S, D)


# revision 26
# speedup vs baseline: 1.0197x; 1.0197x over previous
"""Trainium2 Bass kernel for GQA attention (B=2, S=2048, D=4096, 32 q-heads,
8 kv-heads, head_dim=128, RoPE, causal) distributed over 8 NeuronCores.

Sharding: tensor-parallel over heads for QKV+attention (core c owns q-heads
4c..4c+3 and kv-head c, all sequence positions), then per-head AllToAlls
re-shard the attention output from head-sharded to row-sharded (overlapped
with attention of later heads) so the output projection wo contracts locally;
final output rows are gathered on the host.

Device dataflow per core (identical program on all 8 cores, data differs):
  - stream x^T tiles once; accumulate K^T, V^T (own kv head) and Q^T (4 own
    heads) in PSUM; drain PSUM quickly via ScalarE copies; rotate-half RoPE
    on VectorE (weights are column-permuted on the host so RoPE pairs are
    (i, i+64) within each head); V^T -> V via PE transposes, inline per chunk.
  - causal attention per (head, batch, q-tile of 128), software-pipelined:
    scores in PSUM (PE) -> exp on ScalarE (accumulated row-sum = softmax
    denominator for free; no max subtraction needed at these magnitudes) ->
    normalize P by 1/den (VectorE) -> PE-transpose P -> P^T @ V on PE.
  - AllToAll per head (head-sharded -> row-sharded), overlapped.
  - out^T = wo^T @ attn_out^T for this core's 512 rows.
Host returns out[rows_c, :] = out_c^T.T concatenated over cores.
"""
import sys
sys.path.insert(0, "/opt/trn_rl_repo")
import math
import numpy as np

import concourse.bass as bass
import concourse.bacc as bacc
import concourse.tile as tile
import concourse.mybir as mybir
from concourse.bass_utils import run_bass_kernel_spmd
from concourse.dt import dt

B, S, D = 2, 2048, 4096
HQ, HKV, HD = 32, 8, 128
NC_ = 8                       # cores
ROWS = B * S                  # 4096
RPC = ROWS // NC_             # 512 rows per core
HPC = HQ // NC_               # 4 q-heads per core
QT = S // HD                  # 16 q-tiles per batch
DT_ = 32                      # d-tiles (D/128)
SCALE = 1.0 / math.sqrt(HD)
THETA = 10000.0

F32 = mybir.dt.float32
CDT = mybir.dt.bfloat16       # compute dtype for matmul operands
NP_CDT = dt.np(CDT)

_CACHE = {}


def _build():
    nc = bacc.Bacc("TRN2", target_bir_lowering=False, debug=False,
                   num_devices=NC_)
    AF = mybir.ActivationFunctionType

    # ---- dram I/O (names = in_maps keys) ----
    xT = nc.dram_tensor("xT", [DT_, 128, ROWS], CDT, kind="ExternalInput")
    wq = nc.dram_tensor("wq", [128, DT_ * HPC * HD], CDT, kind="ExternalInput")
    wk = nc.dram_tensor("wk", [128, DT_ * HD], CDT, kind="ExternalInput")
    wv = nc.dram_tensor("wv", [128, DT_ * HD], CDT, kind="ExternalInput")
    wo = nc.dram_tensor("wo", [(DT_ // 8) * HPC, 128, NC_ * 8 * HD], CDT,
                        kind="ExternalInput")
    csa = nc.dram_tensor("csa", [128, S], F32, kind="ExternalInput")  # cos|cos
    csb = nc.dram_tensor("csb", [128, S], F32, kind="ExternalInput")  # -sin|sin
    msk = nc.dram_tensor("msk", [128, HD], F32, kind="ExternalInput")
    idn = nc.dram_tensor("idn", [128, 128], CDT, kind="ExternalInput")
    psw = nc.dram_tensor("psw", [128, 128], CDT, kind="ExternalInput")
    out = nc.dram_tensor("out", [DT_ * 128, RPC], F32, kind="ExternalOutput")

    with tile.TileContext(nc) as tc:
        with tc.tile_pool(name="const", bufs=1) as constp, \
             tc.tile_pool(name="persist", bufs=1) as persist, \
             tc.tile_pool(name="dram", bufs=1, space="DRAM") as dram:
            # constants (cos/sin tables loaded on the scalar queue so they
            # don't delay the first projection matmuls' weight DMAs)
            csa_sb = constp.tile([128, S], F32)
            nc.scalar.dma_start(csa_sb[:], csa[:])
            csb_sb = constp.tile([128, S], F32)
            nc.scalar.dma_start(csb_sb[:], csb[:])
            msk_sb = constp.tile([128, HD], F32)
            nc.scalar.dma_start(msk_sb[:], msk[:])
            idn_sb = constp.tile([128, 128], CDT)
            nc.scalar.dma_start(idn_sb[:], idn[:])
            psw_sb = constp.tile([128, 128], CDT)
            nc.scalar.dma_start(psw_sb[:], psw[:])

            # persistent activations
            k_sb = persist.tile([128, ROWS], CDT)           # K^T (rope'd)
            vn_sb = persist.tile([128, ROWS], CDT)          # V natural tiles
            q_sb = persist.tile([128, HPC * ROWS], CDT)     # Q^T per head
            ao_sb = persist.tile([128, HPC * ROWS], CDT)    # attn_out^T per head

            def _rope(dst, src_sb, swp_ps, pos0, n, tmp_pool):
                """dst[128, n] (SBUF CDT) = rope(src_sb[128, n] SBUF CDT),
                given swp_ps[128, n] (PSUM f32) = half-swapped src (from a PE
                matmul with the swap permutation). Positions pos0.. (one
                batch). All ops are full-tile, partition-aligned:
                  dst = src * [cos|cos] + swapped(src) * [-sin|sin]."""
                ca = csa_sb[:, pos0:pos0 + n]
                cb = csb_sb[:, pos0:pos0 + n]
                t = tmp_pool.tile([128, n], F32, tag="ropetmp")
                nc.vector.tensor_mul(t[:], src_sb[:], ca)
                u = tmp_pool.tile([128, n], F32, tag="ropetmp2")
                nc.vector.tensor_mul(u[:], swp_ps[:], cb)
                nc.vector.tensor_add(dst[:], t[:], u[:])

            # ---------------- phase P: projections ----------------
            with tc.tile_pool(name="wts", bufs=1) as wtp, \
                 tc.tile_pool(name="xs", bufs=6) as xsp, \
                 tc.tile_pool(name="ptmp", bufs=2) as ptmp, \
                 tc.tile_pool(name="drain", bufs=2) as drp, \
                 tc.tile_pool(name="pps", bufs=1, space="PSUM") as pps, \
                 tc.tile_pool(name="miscp", bufs=2, space="PSUM") as miscp:
                # weight loads chunked (8 chunks of 4 d-tiles); chunk 0 lands
                # first so matmuls start immediately, the rest interleave
                # with the first row-chunk's xT stream
                wk_sb = wtp.tile([128, DT_ * HD], CDT)
                wv_sb = wtp.tile([128, DT_ * HD], CDT)
                wq_sb = wtp.tile([128, DT_ * HPC * HD], CDT)

                def load_wchunk(qd):
                    c0, c1 = qd * DT_ * HD // 8, (qd + 1) * DT_ * HD // 8
                    nc.sync.dma_start(wk_sb[:, c0:c1], wk[:, c0:c1])
                    nc.sync.dma_start(wv_sb[:, c0:c1], wv[:, c0:c1])
                    q0_, q1_ = qd * DT_ * HPC * HD // 8, \
                        (qd + 1) * DT_ * HPC * HD // 8
                    nc.sync.dma_start(wq_sb[:, q0_:q1_], wq[:, q0_:q1_])

                load_wchunk(0)

                NCH = ROWS // 512   # 8 row-chunks
                for ch in range(NCH):
                    pos0 = (ch * 512) % S
                    kp = pps.tile([128, 512], F32, tag="kp")
                    vp = pps.tile([128, 512], F32, tag="vp")
                    qp = [pps.tile([128, 512], F32, tag=f"qp{h}",
                                   name=f"qp{h}_{ch}")
                          for h in range(HPC)]
                    for d in range(DT_):
                        xt = xsp.tile([128, 512], CDT, tag="xt")
                        nc.sync.dma_start(
                            xt[:], xT[d, :, ch * 512:(ch + 1) * 512])
                        if ch == 0 and d % 4 == 0 and d // 4 + 1 < 8:
                            load_wchunk(d // 4 + 1)
                        st, sp = (d == 0), (d == DT_ - 1)
                        nc.tensor.matmul(kp[:], wk_sb[:, d * HD:(d + 1) * HD],
                                         xt[:], start=st, stop=sp)
                        nc.tensor.matmul(vp[:], wv_sb[:, d * HD:(d + 1) * HD],
                                         xt[:], start=st, stop=sp)
                        for h in range(HPC):
                            w0 = (d * HPC + h) * HD
                            nc.tensor.matmul(qp[h][:],
                                             wq_sb[:, w0:w0 + HD],
                                             xt[:], start=st, stop=sp)
                    sl = slice(ch * 512, (ch + 1) * 512)
                    # drain PSUM fast via ScalarE copies (cast to bf16),
                    # half-swap on PE, then rope on DVE
                    kf = drp.tile([128, 512], CDT, tag="kf")
                    nc.scalar.copy(kf[:], kp[:])
                    vf = drp.tile([128, 512], CDT, tag="vf")
                    nc.scalar.copy(vf[:], vp[:])   # V^T chunk (bf16)
                    qf = [drp.tile([128, 512], CDT, tag=f"qf{h}",
                                   name=f"qf{h}_{ch}")
                          for h in range(HPC)]
                    for h in range(HPC):
                        nc.scalar.copy(qf[h][:], qp[h][:])
                    ksw = miscp.tile([128, 512], F32, tag="misc",
                                     name=f"ksw_{ch}")
                    nc.tensor.matmul(ksw[:], psw_sb[:], kf[:],
                                     start=True, stop=True)
                    _rope(k_sb[:, sl], kf[:], ksw[:], pos0, 512, ptmp)
                    for h in range(HPC):
                        qsw = miscp.tile([128, 512], F32, tag="misc",
                                         name=f"qsw{h}_{ch}")
                        nc.tensor.matmul(qsw[:], psw_sb[:], qf[h][:],
                                         start=True, stop=True)
                        _rope(q_sb[:, h * ROWS + ch * 512:
                                   h * ROWS + (ch + 1) * 512],
                              qf[h][:], qsw[:], pos0, 512, ptmp)
                    # V^T -> V natural, inline (PE transposes + DVE drain)
                    vt = miscp.tile([128, 512], CDT, tag="misc",
                                    name=f"vt_{ch}")
                    for t in range(4):
                        nc.tensor.transpose(vt[:, t * 128:(t + 1) * 128],
                                            vf[:, t * 128:(t + 1) * 128],
                                            idn_sb[:])
                    nc.vector.tensor_copy(vn_sb[:, sl], vt[:])

            # ---------------- phase A: attention (+ per-head A2A) --------
            a2a_in = dram.tile([HPC, NC_, 128, RPC], CDT)
            a2a_out = dram.tile([HPC, NC_, 128, RPC], CDT)

            with tc.tile_pool(name="att", bufs=3) as att, \
                 tc.tile_pool(name="attd", bufs=6) as attd, \
                 tc.tile_pool(name="sps", bufs=2, space="PSUM") as sps, \
                 tc.tile_pool(name="tps2", bufs=2, space="PSUM") as tps2, \
                 tc.tile_pool(name="ops", bufs=2, space="PSUM") as ops:

                def stage1(h, b, j):
                    """QK -> exp/den -> normalized P (PE work: QK matmuls)."""
                    klen = HD * (j + 1)
                    q0 = h * ROWS + b * S + j * HD
                    qt_ap = q_sb[:, q0:q0 + HD]
                    nkc = (klen + 1023) // 1024
                    p_t = att.tile([128, 2048], CDT, tag="p",
                                   name=f"p_{h}_{b}_{j}")
                    den = attd.tile([128, 4], F32, tag="den",
                                    name=f"den_{h}_{b}_{j}")
                    for kc in range(nkc):
                        k0 = kc * 1024
                        kl = min(1024, klen - k0)
                        sp_ = sps.tile([128, 1024], F32, tag="sp",
                                       name=f"sp_{h}_{b}_{j}_{kc}")
                        for nn in range(0, kl, 512):
                            nw = min(512, kl - nn)
                            nc.tensor.matmul(
                                sp_[:, nn:nn + nw], qt_ap,
                                k_sb[:, b * S + k0 + nn:b * S + k0 + nn + nw],
                                start=True, stop=True)
                        if kc == nkc - 1:
                            nc.vector.tensor_add(
                                sp_[:, kl - HD:kl], sp_[:, kl - HD:kl],
                                msk_sb[:])
                        nc.scalar.activation(
                            p_t[:, k0:k0 + kl], sp_[:, 0:kl],
                            AF.Exp, scale=SCALE,
                            accum_out=den[:, kc:kc + 1])
                    for kc in range(1, nkc):
                        nc.vector.tensor_add(den[:, 0:1], den[:, 0:1],
                                             den[:, kc:kc + 1])
                    rden = attd.tile([128, 1], F32, tag="rden",
                                     name=f"rden_{h}_{b}_{j}")
                    nc.vector.reciprocal(rden[:], den[:, 0:1])
                    nc.vector.tensor_scalar_mul(p_t[:, 0:klen],
                                                p_t[:, 0:klen], rden[:])
                    return p_t

                def stage2(h, b, j, p_t, otg):
                    """P transpose -> PV accumulate -> drain OT group."""
                    klen = HD * (j + 1)
                    q0 = h * ROWS + b * S + j * HD
                    pt_t = att.tile([128, 2048], CDT, tag="pt",
                                    name=f"pt_{h}_{b}_{j}")
                    for g in range(0, j + 1, 4):
                        gw = min(4, j + 1 - g)
                        tp = tps2.tile([128, 512], CDT, tag="tp",
                                       name=f"tp_{h}_{b}_{j}_{g}")
                        for t in range(gw):
                            c0 = (g + t) * HD
                            nc.tensor.transpose(tp[:, t * HD:(t + 1) * HD],
                                                p_t[:, c0:c0 + HD], idn_sb[:])
                        nc.vector.tensor_copy(pt_t[:, g * HD:(g + gw) * HD],
                                              tp[:, 0:gw * HD])
                    jj = j % 4
                    for kt in range(j + 1):
                        nc.tensor.matmul(
                            otg[:, jj * HD:(jj + 1) * HD],
                            vn_sb[:, (b * QT + kt) * HD:(b * QT + kt + 1) * HD],
                            pt_t[:, kt * HD:(kt + 1) * HD],
                            start=(kt == 0), stop=(kt == j))
                    if jj == 3:
                        # drain 4 accumulated OTs at once
                        nc.vector.tensor_copy(
                            ao_sb[:, q0 - 3 * HD:q0 + HD], otg[:])

                for h in range(HPC):
                    pending = None
                    otg = None
                    for b in range(B):
                        for j in range(QT):
                            if j % 4 == 0:
                                otg = ops.tile([128, 512], F32, tag="ot",
                                               name=f"ot_{h}_{b}_{j}")
                            p_t = stage1(h, b, j)
                            if pending is not None:
                                stage2(*pending)
                            pending = (h, b, j, p_t, otg)
                            otg_prev = otg
                    stage2(*pending)
                    # per-head AllToAll, overlaps later heads' attention
                    for r in range(NC_):
                        nc.sync.dma_start(
                            a2a_in[h, r, :, :],
                            ao_sb[:, h * ROWS + r * RPC:
                                  h * ROWS + (r + 1) * RPC])
                    nc.gpsimd.collective_compute(
                        "AllToAll", mybir.AluOpType.bypass,
                        ins=[a2a_in[h].opt()], outs=[a2a_out[h].opt()],
                        replica_groups=[list(range(NC_))],
                    )

            # ---------------- phase O: output projection ----------------
            # od-groups of 8 (one PSUM bank each) x h-passes: the h-pass MMs
            # only depend on collective h, so the first 3/4 of each group's
            # matmuls can run before the last A2A lands.
            NG = DT_ // 8   # 4 od-groups
            with tc.tile_pool(name="osb", bufs=1) as osb, \
                 tc.tile_pool(name="wos", bufs=2) as wos, \
                 tc.tile_pool(name="oout", bufs=3) as oout, \
                 tc.tile_pool(name="opp", bufs=1, space="PSUM") as opp:
                # ao2 loads issued on the gpsimd queue: it is naturally
                # ordered after each collective, so head-3's load cannot
                # block the wo weight loads on the sync queue.
                ao2 = osb.tile([128, DT_ * RPC], CDT)
                for h in range(HPC):
                    for s_ in range(NC_):
                        ct = s_ * HPC + h
                        nc.gpsimd.dma_start(
                            ao2[:, ct * RPC:(ct + 1) * RPC],
                            a2a_out[h, s_, :, :])
                for g in range(NG):
                    ops_ = [opp.tile([128, RPC], F32, tag=f"op{i}",
                                     name=f"op_{g}_{i}") for i in range(8)]
                    for h in range(HPC):
                        # weights for this (group, h): [128, s, od_in_g, 128]
                        wo_sb = wos.tile([128, NC_ * 8 * HD], CDT, tag="wo",
                                         name=f"wo_{g}_{h}")
                        nc.sync.dma_start(wo_sb[:], wo[g * HPC + h, :, :])
                        for s_ in range(NC_):
                            ct = s_ * HPC + h
                            for i in range(8):
                                w0 = (s_ * 8 + i) * HD
                                nc.tensor.matmul(
                                    ops_[i][:], wo_sb[:, w0:w0 + HD],
                                    ao2[:, ct * RPC:(ct + 1) * RPC],
                                    start=(h == 0 and s_ == 0),
                                    stop=(h == HPC - 1 and s_ == NC_ - 1))
                    for i in range(8):
                        od = g * 8 + i
                        o_sb = oout.tile([128, RPC], F32, tag="osb",
                                         name=f"osb_{od}")
                        nc.scalar.copy(o_sb[:], ops_[i][:])
                        nc.sync.dma_start(out[od * 128:(od + 1) * 128, :],
                                          o_sb[:])

    nc.compile()
    return nc


def _host_prep(x, wq, wk, wv, wo):
    perm = np.concatenate([np.arange(0, HD, 2), np.arange(1, HD, 2)])
    x2 = np.ascontiguousarray(x.reshape(ROWS, D).T)        # [D, ROWS]
    xT_r = x2.reshape(DT_, 128, ROWS).astype(NP_CDT)

    wq_p = wq.reshape(D, HQ, HD)[:, :, perm].reshape(D, HQ * HD)
    wk_p = wk.reshape(D, HKV, HD)[:, :, perm].reshape(D, HKV * HD)

    # per-core weight shards in sbuf tile layout [128p, d-tile, cols]
    def tile_rows(w):  # [D, C] -> [128, DT_*C] with blocks (d, c)
        Dd, C = w.shape
        return np.ascontiguousarray(
            w.reshape(DT_, 128, C).transpose(1, 0, 2).reshape(128, DT_ * C))

    wq_cores = []
    wk_cores = []
    wv_cores = []
    for c in range(NC_):
        wqc = wq_p[:, c * HPC * HD:(c + 1) * HPC * HD]     # [D, 512]
        wq_cores.append(tile_rows(wqc).astype(NP_CDT))
        wk_cores.append(tile_rows(
            wk_p[:, c * HD:(c + 1) * HD]).astype(NP_CDT))
        wv_cores.append(tile_rows(
            wv[:, c * HD:(c + 1) * HD]).astype(NP_CDT))

    # wo lhsT tiles grouped by (od-group g, head-slot h):
    # wo_t[g*HPC+h, p, (s*8+i)*128+j] = wo[(s*HPC+h)*128+p, (g*8+i)*128+j]
    wo_t = np.ascontiguousarray(
        wo.reshape(NC_, HPC, 128, DT_ // 8, 8, 128)
        .transpose(3, 1, 2, 0, 4, 5)
        .reshape((DT_ // 8) * HPC, 128, NC_ * 8 * 128)).astype(NP_CDT)

    inv = 1.0 / (THETA ** (np.arange(0, HD, 2, dtype=np.float64) / HD))
    ang = np.arange(S, dtype=np.float64)[:, None] * inv[None, :]
    cosT = np.cos(ang).T
    sinT = np.sin(ang).T
    csa = np.concatenate([cosT, cosT], axis=0).astype(np.float32)
    csb = np.concatenate([-sinT, sinT], axis=0).astype(np.float32)

    m = np.where(np.arange(HD)[None, :] > np.arange(HD)[:, None],
                 np.float32(-1e9), np.float32(0.0)).astype(np.float32)
    ident = np.eye(128, dtype=np.float32).astype(NP_CDT)
    # half-swap permutation: out[i] = in[(i+64)%128]  (out = psw.T @ in)
    pswap = np.zeros((128, 128), np.float32)
    pswap[(np.arange(128) + 64) % 128, np.arange(128)] = 1.0
    pswap = pswap.astype(NP_CDT)

    return (xT_r, wq_cores, wk_cores, wv_cores, wo_t, csa, csb, m, ident,
            pswap)


def kernel(x, wq, wk, wv, wo):
    if "nc" not in _CACHE:
        _CACHE["nc"] = _build()
    nc = _CACHE["nc"]

    xT_r, wq_c, wk_c, wv_c, wo_t, csa, csb, m, ident, pswap = _host_prep(
        np.asarray(x, np.float32), np.asarray(wq, np.float32),
        np.asarray(wk, np.float32), np.asarray(wv, np.float32),
        np.asarray(wo, np.float32))

    in_maps = []
    for c in range(NC_):
        in_maps.append({
            "xT": xT_r, "wq": wq_c[c], "wk": wk_c[c], "wv": wv_c[c],
            "wo": wo_t, "csa": csa, "csb": csb, "msk": m, "idn": ident,
            "psw": pswap,
        })
    res = run_bass_kernel_spmd(nc, in_maps, core_ids=list(range(NC_)))
    _CACHE["last_results"] = res

    outp = np.empty((ROWS, D), np.float32)
    for c in range(NC_):
        outp[c * RPC:(c + 1) * RPC, :] = res.results[c]["out"].T
    return outp.reshape(B, S, D)
